# revision 1
# baseline (speedup 1.0000x reference)
"""Multi-head attention (B=2, T=2048, D=1024, H=16, dk=64) on 8 trn2 cores.

Sharding: core c -> (batch b = c//4, head-group g = c%4 of 4 heads).
Each core computes its head-group's Q/K/V projections (column-sliced),
attention for 4 heads, and a partial output projection (row-sliced Wo).
Host sums the 4 partials per batch (the "all-reduce") and adds bo.

Device-side layout trick: the host pre-transposes q/k/v to x^T [D, T], so
  Q^T = (Wq_g)^T @ x^T   (lhsT = Wq natural, rhs = x^T)    -> [256, T]
  K^T likewise                                              -> [256, T]
  V   = x @ Wv_g         (lhsT = x^T, rhs = Wv natural)     -> [T, 256]
i.e. zero on-device transposes. Scores are computed transposed,
S^T[k, q] = K_h Q_h^T, softmax needs no max subtraction (inputs are
N(0,1)-scaled; |S|/8 < ~7 so exp cannot overflow), and the softmax
denominator falls out of the P@V matmul for free via a ones-column
appended to V (M=65). All matmuls run float32r (fp32 data, full PE rate
at N>=256; measured rel err ~1.5e-4 on K=1024 dots).
"""
import os
import sys

for _p in ("/opt/trn_rl_repo", "/root/.axon_site/_ro/trn_rl_repo"):
    if os.path.isdir(_p) and _p not in sys.path:
        sys.path.append(_p)

from contextlib import ExitStack

import ml_dtypes
import numpy as np

import concourse.tile as tile
from concourse import bacc, mybir
from concourse.bass_utils import run_bass_kernel_spmd

F32 = mybir.dt.float32
F32R = mybir.dt.float32r
BF16 = mybir.dt.bfloat16
EXP = mybir.ActivationFunctionType.Exp

D = 1024          # d_model
T = 2048          # sequence length
HG = 4            # heads per core
DK = 64           # head dim
GC = HG * DK      # group cols = 256
DC = D // 128     # 8 d-chunks
KT = T // 128     # 16 key tiles
QH = 2            # q halves
QW = T // QH      # 1024 q-half width
VB = HG * (DK + 1)  # V_aug block: 4 heads x (64 vals + ones col) = 260
N_CORES = 8

_NC_CACHE = {}


def _build(with_qkv_bias: bool):
    nc = bacc.Bacc("TRN2", target_bir_lowering=False, debug=False,
                   num_devices=N_CORES)

    xqT = nc.dram_tensor("xqT", [D, T], BF16, kind="ExternalInput")
    xkT = nc.dram_tensor("xkT", [D, T], BF16, kind="ExternalInput")
    xvT = nc.dram_tensor("xvT", [D, T], BF16, kind="ExternalInput")
    wq = nc.dram_tensor("wq", [D, GC], BF16, kind="ExternalInput")
    wk = nc.dram_tensor("wk", [D, GC], BF16, kind="ExternalInput")
    wv = nc.dram_tensor("wv", [D, GC], BF16, kind="ExternalInput")
    wo = nc.dram_tensor("wo", [GC, D], F32R, kind="ExternalInput")
    if with_qkv_bias:
        bqkv = nc.dram_tensor("bqkv", [3, GC], BF16, kind="ExternalInput")
    out = nc.dram_tensor("out_partial", [T, D], BF16, kind="ExternalOutput")

    with tile.TileContext(nc) as tc, ExitStack() as ctx:
        # Persistent SBUF pools.
        wpool = ctx.enter_context(tc.tile_pool(name="w", bufs=1))
        cpool = ctx.enter_context(tc.tile_pool(name="const", bufs=1))
        qkpool = ctx.enter_context(tc.tile_pool(name="qk", bufs=1))
        vaugpool = ctx.enter_context(tc.tile_pool(name="vaug", bufs=1))
        ctxpool = ctx.enter_context(tc.tile_pool(name="ctxT", bufs=1))
        espool = ctx.enter_context(tc.tile_pool(name="es", bufs=4))
        xin8 = ctx.enter_context(tc.tile_pool(name="xin8", bufs=8))

        # ---- weights to SBUF (d-chunk c of W at cols c*GC) ----
        wq_sb = wpool.tile([128, DC * GC], BF16, name="wq_sb")
        wk_sb = wpool.tile([128, DC * GC], BF16, name="wk_sb")
        wv_sb = wpool.tile([128, DC * GC], BF16, name="wv_sb")
        wo_sb = wpool.tile([128, 2 * D], F32R, name="wo_sb")
        # xq stream + wq first so the first projection matmul can start ASAP
        xin_q = [xin8.tile([128, T], BF16, name=f"xin_0_{d}", tag="xin8")
                 for d in range(DC)]
        nc.sync.dma_start(xin_q[0][:, :], xqT[0:128, :])
        for c in range(DC):
            nc.sync.dma_start(wq_sb[:, c * GC:(c + 1) * GC],
                              wq[c * 128:(c + 1) * 128, :])
        for d in range(1, DC):
            nc.sync.dma_start(xin_q[d][:, :], xqT[d * 128:(d + 1) * 128, :])
        for t, dram in ((wk_sb, wk), (wv_sb, wv)):
            for c in range(DC):
                nc.sync.dma_start(t[:, c * GC:(c + 1) * GC],
                                  dram[c * 128:(c + 1) * 128, :])
        for j in range(2):
            nc.sync.dma_start(wo_sb[:, j * D:(j + 1) * D],
                              wo[j * 128:(j + 1) * 128, :])
        dummy_bf = cpool.tile([128, 512], BF16, name="dummy_bf")
        ones_st = cpool.tile([128, 512], F32, name="ones_st")
        nc.vector.memset(ones_st[:, :], 1.0)
        nc.vector.tensor_copy(dummy_bf[:, :], ones_st[:, :])
        ones_bf = cpool.tile([1, 512], BF16, name="ones_bf")
        nc.vector.tensor_copy(ones_bf[:, :], ones_st[0:1, :])
        if with_qkv_bias:
            b_sb = cpool.tile([3, GC], BF16, name="b_sb")
            nc.sync.dma_start(b_sb[:, :], bqkv[:, :])

        qt_sb = [qkpool.tile([128, T], BF16, name=f"qt_sb{m}") for m in range(2)]
        kt_sb = [qkpool.tile([128, T], BF16, name=f"kt_sb{m}") for m in range(2)]
        vaug = vaugpool.tile([128, KT * VB], BF16, name="vaug")
        ctx_sb = [ctxpool.tile([128, T], F32R, name=f"ctx_sb{m}") for m in range(2)]

        # ---- Q^T / K^T projections (d-outer, streaming x^T chunks) ----
        with tc.tile_pool(name="pp_proj", bufs=1, space="PSUM") as pp_proj:
            pwarm = pp_proj.tile([128, 512], F32, name="pwarm", tag="pp_m0")
            for r in range(16):
                nc.tensor.matmul(pwarm[:, :], lhsT=dummy_bf[:, 0:128],
                                 rhs=dummy_bf[:, :], start=True, stop=True)
            for w_sb, xT, dst, brow in ((wq_sb, xqT, qt_sb, 0),
                                        (wk_sb, xkT, kt_sb, 1)):
                ps = [pp_proj.tile([128, T], F32, name=f"pp_m{m}", tag=f"pp_m{m}")
                      for m in range(2)]
                if brow == 0:
                    xin = xin_q
                else:
                    xin = [xin8.tile([128, T], BF16, name=f"xin_{brow}_{d}",
                                     tag="xin8") for d in range(DC)]
                for d in range(DC):
                    if brow != 0:
                        nc.sync.dma_start(xin[d][:, :], xT[d * 128:(d + 1) * 128, :])
                    for m in range(2):
                        for q4 in range(4):
                            nc.tensor.matmul(
                                ps[m][:, q4 * 512:(q4 + 1) * 512],
                                lhsT=w_sb[:, d * GC + m * 128:d * GC + (m + 1) * 128],
                                rhs=xin[d][:, q4 * 512:(q4 + 1) * 512],
                                start=(d == 0),
                                stop=(d == DC - 1 and not with_qkv_bias),
                            )
                if with_qkv_bias:
                    for m in range(2):
                        for q4 in range(4):
                            nc.tensor.matmul(
                                ps[m][:, q4 * 512:(q4 + 1) * 512],
                                lhsT=b_sb[brow:brow + 1, m * 128:(m + 1) * 128],
                                rhs=ones_bf[:, :],
                                start=False,
                                stop=True,
                            )
                for m in range(2):
                    nc.vector.tensor_copy(dst[m][:, :], ps[m][:, :])

        # ---- V projection (kt-outer; full x_v^T resident) ----
        # V_aug: kt block of VB=260 cols, head h at h*65 (64 vals + ones col)
        # so the P@V matmul's 65th output row is the softmax denominator.
        with tc.tile_pool(name="vx", bufs=1) as vxpool, \
                tc.tile_pool(name="pp_v", bufs=2, space="PSUM") as pp_v:
            xv_sb = vxpool.tile([128, DC * T], BF16, name="xv_sb")
            for d in range(DC):
                nc.sync.dma_start(xv_sb[:, d * T:(d + 1) * T],
                                  xvT[d * 128:(d + 1) * 128, :])
            nc.vector.tensor_copy(
                vaug[:, :].rearrange("p (k h e) -> p k h e", k=KT, h=HG)[:, :, :, 64:65],
                ones_st[:, 0:KT * HG].rearrange("p (k h e) -> p k h e", k=KT, h=HG),
            )
            for kt in range(KT):
                vps = pp_v.tile([128, GC], F32, name="vps", tag="pp_v")
                for d in range(DC):
                    nc.tensor.matmul(
                        vps[:, :],
                        lhsT=xv_sb[:, d * T + kt * 128:d * T + (kt + 1) * 128],
                        rhs=wv_sb[:, d * GC:(d + 1) * GC],
                        start=(d == 0),
                        stop=(d == DC - 1 and not with_qkv_bias),
                    )
                if with_qkv_bias:
                    nc.tensor.matmul(
                        vps[:, :],
                        lhsT=ones_bf[:, 0:128],
                        rhs=b_sb[2:3, :],
                        start=False,
                        stop=True,
                    )
                nc.vector.tensor_copy(
                    vaug[:, kt * VB:(kt + 1) * VB]
                    .rearrange("p (h e) -> p h e", h=HG)[:, :, 0:64],
                    vps[:, :].rearrange("p (h dd) -> p h dd", h=HG),
                )

        # ---- attention + normalization ----
        # Head-PAIR blocks: the two heads of a pair issue adjacent row-tiled
        # S-matmuls (lhsT base partitions 0 and 64 -> tile_position row
        # groups) so they run concurrently on the PE and keep the full array
        # active (HAM stays warm). Emission order keeps ACT (exp) saturated:
        # exp_h0(i), exp_h1(i), PV_h0(i), PV_h1(i), S_pair(i+1).
        # Normalization copies ctx out of PSUM immediately (early slot
        # release), then runs reciprocal + gpsimd partition-broadcast +
        # multiply entirely from SBUF off the critical path.
        with tc.tile_pool(name="pp_s", bufs=1, space="PSUM") as pp_s, \
                tc.tile_pool(name="pp_ctx", bufs=1, space="PSUM") as pp_ctx, \
                tc.tile_pool(name="rz", bufs=2) as rzpool, \
                tc.tile_pool(name="ub", bufs=2) as ubpool, \
                tc.tile_pool(name="bc", bufs=2) as bcpool:
            blocks = [(qh, hp) for qh in range(QH) for hp in range(2)]
            seq = [(bi, kt) for bi in range(len(blocks)) for kt in range(KT)]
            sps_tiles = {}
            cps_tiles = {}

            def emit_s_pair(i):
                bi, kt = seq[i]
                qh, hp = blocks[bi]
                q0 = qh * QW
                t0 = pp_s.tile([128, QW], F32, name="sps0", tag="pp_s0")
                t1 = pp_s.tile([128, QW], F32, name="sps1", tag="pp_s1")
                sps_tiles[i] = (t0, t1)
                for sc in range(QW // 512):
                    for hi, t in ((0, t0), (1, t1)):
                        off = hi * 64
                        nc.tensor.matmul(
                            t[:, sc * 512:(sc + 1) * 512],
                            lhsT=kt_sb[hp][off:off + 64, kt * 128:(kt + 1) * 128],
                            rhs=qt_sb[hp][off:off + 64,
                                          q0 + sc * 512:q0 + (sc + 1) * 512],
                        )

            warm = pp_s.tile([128, 512], F32, name="warm", tag="pp_s0")
            for r in range(16):
                nc.tensor.matmul(
                    warm[:, :],
                    lhsT=qt_sb[0][:, 0:128],
                    rhs=kt_sb[0][:, 0:512],
                    start=True,
                    stop=True,
                )
            emit_s_pair(0)
            for i, (bi, kt) in enumerate(seq):
                qh, hp = blocks[bi]
                q0 = qh * QW
                if kt == 0:
                    cps_tiles[bi] = (
                        pp_ctx.tile([65, QW], F32, name="cps0", tag="pp_ctx0"),
                        pp_ctx.tile([65, QW], F32, name="cps1", tag="pp_ctx1"),
                    )
                cpair = cps_tiles[bi]
                spair = sps_tiles.pop(i)
                es_pair = []
                for hi in range(2):
                    es = espool.tile([128, QW], BF16, name=f"es{hi}", tag="es")
                    nc.scalar.activation(es[:, :], spair[hi][:, :], EXP, scale=0.125)
                    es_pair.append(es)
                for hi in range(2):
                    h = 2 * hp + hi
                    for sc in range(QW // 512):
                        nc.tensor.matmul(
                            cpair[hi][:, sc * 512:(sc + 1) * 512],
                            lhsT=vaug[:, kt * VB + h * 65:kt * VB + h * 65 + 65],
                            rhs=es_pair[hi][:, sc * 512:(sc + 1) * 512],
                            start=(kt == 0),
                            stop=(kt == KT - 1),
                        )
                    if hi == 0 and i + 1 < len(seq):
                        # emit the next S-pair between the two PV halves: its
                        # h0 matmuls only need exp_h0(i)'s slot, so the next
                        # exp can start while exp_h1(i) is still running
                        emit_s_pair(i + 1)
                if kt == KT - 1 and bi < len(blocks) - 1:
                    # keep the PE busy across the block boundary so HAM
                    # doesn't re-throttle the clock
                    bwarm = pp_s.tile([128, 512], F32, name="bwarm", tag="pp_s0")
                    for r in range(4):
                        nc.tensor.matmul(bwarm[:, :], lhsT=dummy_bf[:, 0:128],
                                         rhs=dummy_bf[:, :], start=True, stop=True)
                if kt == KT - 1:
                    # interleave the two heads' chains so GpSimd broadcast
                    # overlaps Vector work
                    zrows, ubs, rzs, bsbs = [], [], [], []
                    last = bi == len(blocks) - 1
                    # per-head: evacuate ctx + Z together so each cps slot
                    # releases as early as possible (next block's PV waits)
                    for hi in range(2):
                        if not last:
                            ub = ubpool.tile([64, QW], F32R, name=f"ub{hi}",
                                             tag=f"ub{hi}")
                            nc.vector.tensor_copy(ub[:, :], cpair[hi][0:64, :])
                            ubs.append(ub)
                        zrow = rzpool.tile([1, QW], F32, name=f"zrow{hi}",
                                           tag=f"zrow{hi}")
                        nc.vector.tensor_copy(zrow[:, :], cpair[hi][64:65, :])
                        zrows.append(zrow)
                    if last:
                        for hi in range(2):
                            rz = rzpool.tile([1, QW], F32, name=f"rz{hi}",
                                             tag=f"rz{hi}")
                            rzs.append(rz)
                            bsb = bcpool.tile([64, QW], F32, name=f"bsb{hi}",
                                              tag=f"bc{hi}")
                            bsbs.append(bsb)
                        for sc in range(2):
                            sl = slice(sc * 512, (sc + 1) * 512)
                            for hi in range(2):
                                with nc.allow_low_precision(reason="recip ok"):
                                    nc.vector.reciprocal_approx_fast(
                                        rzs[hi][0:1, sl], zrows[hi][0:1, sl])
                                nc.gpsimd.partition_broadcast(
                                    bsbs[hi][:, sl], rzs[hi][0:1, sl])
                    else:
                        for hi in range(2):
                            rz = rzpool.tile([1, QW], F32, name=f"rz{hi}",
                                             tag=f"rz{hi}")
                            with nc.allow_low_precision(reason="~18-bit recip ok"):
                                nc.vector.reciprocal_approx_fast(rz[:, :],
                                                                 zrows[hi][:, :])
                            rzs.append(rz)
                            bsb = bcpool.tile([64, QW], F32, name=f"bsb{hi}",
                                              tag=f"bc{hi}")
                            nc.gpsimd.partition_broadcast(bsb[:, :], rz[:, :])
                            bsbs.append(bsb)
                    if last:
                        # last block: multiply straight from PSUM, in halves,
                        # so outproj tiles unblock incrementally
                        for sc in range(2):
                            sl = slice(sc * 512, (sc + 1) * 512)
                            for hi in range(2):
                                nc.vector.tensor_mul(
                                    ctx_sb[hp][hi * 64:hi * 64 + 64,
                                               q0 + sc * 512:q0 + (sc + 1) * 512],
                                    cpair[hi][0:64, sl],
                                    bsbs[hi][:, sl],
                                )
                    else:
                        for hi in range(2):
                            nc.vector.tensor_mul(
                                ctx_sb[hp][hi * 64:hi * 64 + 64, q0:q0 + QW],
                                ubs[hi][:, :],
                                bsbs[hi][:, :],
                            )
                    del cps_tiles[bi]

        # ---- output projection: out[q, :] = ctx[q, :] @ Wo_g (partial) ----
        with tc.tile_pool(name="pp_o", bufs=6, space="PSUM") as pp_o, \
                tc.tile_pool(name="osb", bufs=4) as opool:
            # keep the PE warm across the attention->outproj transition
            owarm = pp_o.tile([128, 512], F32, name="owarm", tag="pp_o")
            for r in range(8):
                nc.tensor.matmul(owarm[:, :], lhsT=dummy_bf[:, 0:128],
                                 rhs=dummy_bf[:, :], start=True, stop=True)
            for qt in range(T // 128):
                osb = opool.tile([128, D], BF16, name="osb", tag="osb")
                for n2 in range(2):
                    ops = pp_o.tile([128, 512], F32, name="ops", tag="pp_o")
                    for j in range(2):
                        nc.tensor.matmul(
                            ops[:, :],
                            lhsT=ctx_sb[j][:, qt * 128:(qt + 1) * 128],
                            rhs=wo_sb[:, j * D + n2 * 512:j * D + (n2 + 1) * 512],
                            start=(j == 0),
                            stop=(j == 1),
                        )
                    half = osb[:, n2 * 512:(n2 + 1) * 512]
                    if (2 * qt + n2) % 2 == 0:
                        nc.vector.tensor_copy(half, ops[:, :])
                    else:
                        nc.scalar.copy(half, ops[:, :])
                nc.sync.dma_start(out[qt * 128:(qt + 1) * 128, :], osb[:, :])

    nc.compile()
    return nc


def kernel(q, k, v, Wq, bq, Wk, bk, Wv, bv, Wo, bo, **extra):
    q = np.asarray(q, np.float32)
    k = np.asarray(k, np.float32)
    v = np.asarray(v, np.float32)
    Wq, Wk, Wv, Wo = (np.asarray(a, np.float32) for a in (Wq, Wk, Wv, Wo))
    bq, bk, bv, bo = (np.asarray(a, np.float32) for a in (bq, bk, bv, bo))
    B = q.shape[0]
    assert q.shape == (B, T, D)

    with_qkv_bias = bool(np.any(bq) or np.any(bk) or np.any(bv))
    if with_qkv_bias not in _NC_CACHE:
        _NC_CACHE[with_qkv_bias] = _build(with_qkv_bias)
    nc = _NC_CACHE[with_qkv_bias]

    bf = ml_dtypes.bfloat16
    xT = {}
    for b in range(B):
        xT[("q", b)] = np.ascontiguousarray(q[b].T.astype(bf))
        xT[("k", b)] = np.ascontiguousarray(k[b].T.astype(bf))
        xT[("v", b)] = np.ascontiguousarray(v[b].T.astype(bf))

    in_maps = []
    for c in range(N_CORES):
        b, g = c // HG, c % HG
        sl = slice(g * GC, (g + 1) * GC)
        m = {
            "xqT": xT[("q", b)],
            "xkT": xT[("k", b)],
            "xvT": xT[("v", b)],
            "wq": np.ascontiguousarray(Wq[:, sl].astype(bf)),
            "wk": np.ascontiguousarray(Wk[:, sl].astype(bf)),
            "wv": np.ascontiguousarray(Wv[:, sl].astype(bf)),
            "wo": np.ascontiguousarray(Wo[sl, :]),
        }
        if with_qkv_bias:
            m["bqkv"] = np.ascontiguousarray(np.stack([bq[sl], bk[sl], bv[sl]]).astype(bf))
        in_maps.append(m)

    trace = bool(int(os.environ.get("MHA_TRACE", "0")))
    res = run_bass_kernel_spmd(nc, in_maps, list(range(N_CORES)), trace=trace)
    if trace:
        kernel.last_results = res

    out = np.empty((B, T, D), np.float32)
    for b in range(B):
        acc = res.results[b * HG]["out_partial"].astype(np.float32)
        for g in range(1, HG):
            acc = acc + res.results[b * HG + g]["out_partial"]
        out[b] = acc + bo[None, :]
    return out



# revision 7
# speedup vs baseline: 1.1067x; 1.1067x over previous
"""Multi-head attention (B=2, T=2048, D=1024, H=16, dk=64) on 8 trn2 cores.

Sharding: core c -> (batch b = c//4, head-group g = c%4 of 4 heads).
Each core computes its head-group's Q/K/V projections (column-sliced),
attention for 4 heads, and a partial output projection (row-sliced Wo).
Host sums the 4 partials per batch (the "all-reduce") and adds bo.

Device-side layout: host pre-transposes q/k/v to x^T [D, T], so
  Q^T = (Wq_g)^T @ x^T    -> [256, T]     (zero on-device transposes)
  K^T likewise            -> [256, T]
  V   = x @ Wv_g          -> [T, 256]
Scores are computed transposed, S^T[k, q] = K_h Q_h^T; softmax needs no
max subtraction (N(0,1)-scaled inputs; |S|/8 < ~7), and the denominator
falls out of the P@V matmul via a ones-column appended to V (M=65).

v2 (just-in-time single-head blocks): 8 blocks of (q-half, head) x 16
key-tiles; one [128,1024] S^T psum tile and one FD=1024 exp per
iteration. PSUM: S double-buffered (4 banks) + ctx accumulator (2) +
background-projection scratch (2) = 8 exactly. All input DMAs issue
upfront in deadline order into dedicated [128,512] tiles; only a
minimal Q/K prefix runs before the attention loop; all remaining
projection chunks, V kt-tiles, and the first q-half's out-proj are
injected between iterations as <=1.7us background granules. The exp
activation table is preloaded during the initial DMA wait.
"""
import os
import sys

for _p in ("/opt/trn_rl_repo", "/root/.axon_site/_ro/trn_rl_repo"):
    if os.path.isdir(_p) and _p not in sys.path:
        sys.path.append(_p)

from contextlib import ExitStack

import ml_dtypes
import numpy as np

import concourse.tile as tile
from concourse import bacc, mybir
from concourse.bass_utils import run_bass_kernel_spmd

F32 = mybir.dt.float32
F32R = mybir.dt.float32r
BF16 = mybir.dt.bfloat16
EXP = mybir.ActivationFunctionType.Exp

D = 1024          # d_model
T = 2048          # sequence length
HG = 4            # heads per core
DK = 64           # head dim
GC = HG * DK      # group cols = 256
DC = D // 128     # 8 d-chunks
KT = T // 128     # 16 key tiles
QH = 2            # q halves
QW = T // QH      # 1024 q-half width
VB = HG * (DK + 1)  # V_aug block: 4 heads x (64 vals + ones col) = 260
KB = 4            # 512-wide key/q column blocks
N_CORES = 8

_NC_CACHE = {}


def _build(with_qkv_bias: bool):
    nc = bacc.Bacc("TRN2", target_bir_lowering=False, debug=False,
                   num_devices=N_CORES)

    # host-repacked layouts: x tensors as [KB*128, DC*512] (key/col-block
    # kb at rows kb*128, d-chunk dd at cols dd*512) so each 512-block loads
    # with ONE plain 2D dma_start; weights as [128, DC*GC] chunk-major.
    xqT = nc.dram_tensor("xqT", [KB * 128, DC * 512], BF16, kind="ExternalInput")
    xkT = nc.dram_tensor("xkT", [KB * 128, DC * 512], BF16, kind="ExternalInput")
    xvT = nc.dram_tensor("xvT", [KB * 128, DC * 512], BF16, kind="ExternalInput")
    wq = nc.dram_tensor("wq", [128, DC * GC], BF16, kind="ExternalInput")
    wk = nc.dram_tensor("wk", [128, DC * GC], BF16, kind="ExternalInput")
    wv = nc.dram_tensor("wv", [128, DC * GC], BF16, kind="ExternalInput")
    wo = nc.dram_tensor("wo", [128, 2 * D], BF16, kind="ExternalInput")
    if with_qkv_bias:
        bqkv = nc.dram_tensor("bqkv", [3, GC], BF16, kind="ExternalInput")
    out = nc.dram_tensor("out_partial", [T, D], BF16, kind="ExternalOutput")

    with tile.TileContext(nc) as tc, ExitStack() as ctx:
        wpool = ctx.enter_context(tc.tile_pool(name="w", bufs=1))
        cpool = ctx.enter_context(tc.tile_pool(name="const", bufs=1))
        qkpool = ctx.enter_context(tc.tile_pool(name="qk", bufs=1))
        vaugpool = ctx.enter_context(tc.tile_pool(name="vaug", bufs=1))
        ctxpool = ctx.enter_context(tc.tile_pool(name="ctxT", bufs=1))
        espool = ctx.enter_context(tc.tile_pool(name="es", bufs=6))
        xpool = ctx.enter_context(tc.tile_pool(name="xin", bufs=1))

        wq_sb = wpool.tile([128, DC * GC], BF16, name="wq_sb")
        wk_sb = wpool.tile([128, DC * GC], BF16, name="wk_sb")
        wv_sb = wpool.tile([128, DC * GC], BF16, name="wv_sb")
        wo_sb = wpool.tile([128, 2 * D], BF16, name="wo_sb")

        xq_t = [xpool.tile([128, DC * 512], BF16, name=f"xq_{b}")
                for b in range(KB)]
        xk_t = [xpool.tile([128, DC * 512], BF16, name=f"xk_{b}")
                for b in range(KB)]
        xv_t = [xpool.tile([128, DC * 512], BF16, name=f"xv_{b}")
                for b in range(KB)]

        def dma_in(tiles, dram, b):
            nc.sync.dma_start(tiles[b][:, :],
                              dram[b * 128:(b + 1) * 128, :])

        # ---- all input DMAs upfront, in deadline order ----
        nc.sync.dma_start(wq_sb[:, :], wq[:, :])
        nc.sync.dma_start(wk_sb[:, :], wk[:, :])
        if with_qkv_bias:
            b_sb = cpool.tile([3, GC], BF16, name="b_sb")
            nc.sync.dma_start(b_sb[:, :], bqkv[:, :])
        dma_in(xq_t, xqT, 0)
        dma_in(xq_t, xqT, 1)
        dma_in(xk_t, xkT, 0)
        nc.sync.dma_start(wv_sb[:, :], wv[:, :])
        dma_in(xv_t, xvT, 0)
        dma_in(xk_t, xkT, 1)
        dma_in(xv_t, xvT, 1)
        dma_in(xk_t, xkT, 2)
        dma_in(xv_t, xvT, 2)
        dma_in(xk_t, xkT, 3)
        dma_in(xv_t, xvT, 3)
        dma_in(xq_t, xqT, 2)
        dma_in(xq_t, xqT, 3)
        nc.sync.dma_start(wo_sb[:, :], wo[:, :])

        # ---- constants + exp-table preload (during DMA wait) ----
        dummy_bf = cpool.tile([128, 512], BF16, name="dummy_bf")
        ones_st = cpool.tile([128, 512], F32, name="ones_st")
        nc.vector.memset(ones_st[:, :], 1.0)
        nc.vector.tensor_copy(dummy_bf[:, :], ones_st[:, :])
        ones_bf = cpool.tile([1, 512], BF16, name="ones_bf")
        nc.vector.tensor_copy(ones_bf[:, :], ones_st[0:1, :])
        tbl_warm = cpool.tile([1, 8], BF16, name="tbl_warm")
        nc.scalar.activation(tbl_warm[:, :], ones_st[0:1, 0:8], EXP)

        qt_sb = [qkpool.tile([128, T], BF16, name=f"qt_sb{m}") for m in range(2)]
        kt_sb = [qkpool.tile([128, T], BF16, name=f"kt_sb{m}") for m in range(2)]
        vaug_t = [vaugpool.tile([128, VB], BF16, name=f"vaug{kt}")
                  for kt in range(KT)]
        ctx_sb = [ctxpool.tile([128, T], BF16, name=f"ctx_sb{m}") for m in range(2)]

        for kt in range(KT):
            nc.vector.memset(vaug_t[kt][:, :], 1.0)

        with tc.tile_pool(name="pp_s", bufs=1, space="PSUM") as pp_s, \
                tc.tile_pool(name="pp_ctx", bufs=1, space="PSUM") as pp_ctx, \
                tc.tile_pool(name="pp_bg", bufs=2, space="PSUM") as pp_bg, \
                tc.tile_pool(name="rz", bufs=2) as rzpool, \
                tc.tile_pool(name="ub", bufs=2) as ubpool, \
                tc.tile_pool(name="bc", bufs=2) as bcpool, \
                tc.tile_pool(name="osb", bufs=4) as opool:

            # ---- background granules ----
            def qkproj(which, m, cb):
                w_sb, dst, brow = ((wq_sb, qt_sb, 0) if which == "q"
                                   else (wk_sb, kt_sb, 1))
                xmap = xq_t if which == "q" else xk_t
                ps = pp_bg.tile([128, 512], F32, name="qkps", tag="bg")
                for dd in range(DC):
                    nc.tensor.matmul(
                        ps[:, :],
                        lhsT=w_sb[:, dd * GC + m * 128:dd * GC + (m + 1) * 128],
                        rhs=xmap[cb][:, dd * 512:(dd + 1) * 512],
                        start=(dd == 0),
                        stop=(dd == DC - 1 and not with_qkv_bias),
                    )
                if with_qkv_bias:
                    nc.tensor.matmul(
                        ps[:, :],
                        lhsT=b_sb[brow:brow + 1, m * 128:(m + 1) * 128],
                        rhs=ones_bf[:, :],
                        start=False, stop=True,
                    )
                nc.vector.tensor_copy(
                    dst[m][:, cb * 512:(cb + 1) * 512], ps[:, :])

            def vproj(kt):
                kb, sub = kt // 4, kt % 4
                ps = pp_bg.tile([128, GC], F32, name="vps", tag="bg")
                for dd in range(DC):
                    nc.tensor.matmul(
                        ps[:, :],
                        lhsT=xv_t[kb][:, dd * 512 + sub * 128:dd * 512 + (sub + 1) * 128],
                        rhs=wv_sb[:, dd * GC:(dd + 1) * GC],
                        start=(dd == 0),
                        stop=(dd == DC - 1 and not with_qkv_bias),
                    )
                if with_qkv_bias:
                    nc.tensor.matmul(
                        ps[:, :],
                        lhsT=ones_bf[:, 0:128],
                        rhs=b_sb[2:3, :],
                        start=False, stop=True,
                    )
                for h in range(HG):
                    nc.vector.tensor_copy(
                        vaug_t[kt][:, h * 65:h * 65 + 64],
                        ps[:, h * 64:(h + 1) * 64],
                    )

            osb_tiles = {}

            def oproj(qt, n2, on_act=False):
                if n2 == 0:
                    osb_tiles[qt] = opool.tile([128, D], BF16, name="osb",
                                               tag="osb")
                osb = osb_tiles[qt]
                ps = pp_bg.tile([128, 512], F32, name="ops", tag="bg")
                for j in (1, 0):
                    nc.tensor.matmul(
                        ps[:, :],
                        lhsT=ctx_sb[j][:, qt * 128:(qt + 1) * 128],
                        rhs=wo_sb[:, j * D + n2 * 512:j * D + (n2 + 1) * 512],
                        start=(j == 1),
                        stop=(j == 0),
                    )
                half = osb[:, n2 * 512:(n2 + 1) * 512]
                if on_act:
                    nc.scalar.copy(half, ps[:, :])
                else:
                    nc.vector.tensor_copy(half, ps[:, :])
                if n2 == 1:
                    nc.sync.dma_start(out[qt * 128:(qt + 1) * 128, :],
                                      osb[:, :])
                    del osb_tiles[qt]

            # background schedule: emit bg[i] after iteration i's work.
            # Deadlines: V kt_j before PV(iter j of block 0); K(0,kb)
            # before S(4*kb) is emitted (iter 4*kb-1); K(1,*) before
            # block 2 (iter 32); Q(*,2/3) before block 4 (iter 64);
            # out-proj q-half 0 after block 3's norm (iter 63).
            bg = {i: [] for i in range(128)}
            for j in range(2, 16):
                bg[j - 2] += [lambda j=j: vproj(j)]
            bg[2] += [lambda: qkproj("k", 0, 1)]
            bg[6] += [lambda: qkproj("k", 0, 2)]
            bg[10] += [lambda: qkproj("k", 0, 3)]
            bg[16] += [lambda: qkproj("q", 1, 0)]
            bg[19] += [lambda: qkproj("q", 1, 1)]
            bg[22] += [lambda: qkproj("k", 1, 0)]
            bg[24] += [lambda: qkproj("k", 1, 1)]
            bg[26] += [lambda: qkproj("k", 1, 2)]
            bg[28] += [lambda: qkproj("k", 1, 3)]
            bg[40] += [lambda: qkproj("q", 0, 2)]
            bg[44] += [lambda: qkproj("q", 0, 3)]
            bg[48] += [lambda: qkproj("q", 1, 2)]
            bg[52] += [lambda: qkproj("q", 1, 3)]
            for idx in range(16):
                bg[64 + idx] += [
                    (lambda qt=idx // 2, n2=idx % 2: oproj(qt, n2))]

            # ---- PE prefix ----
            warm = pp_bg.tile([128, 512], F32, name="warm", tag="bg")
            for r in range(16):
                nc.tensor.matmul(warm[:, :], lhsT=dummy_bf[:, 0:128],
                                 rhs=dummy_bf[:, :], start=True, stop=True)
            qkproj("q", 0, 0)
            qkproj("q", 0, 1)
            qkproj("k", 0, 0)

            # ---- attention loop: 8 single-head blocks x 16 kt ----
            blocks = [(qh, h) for qh in range(QH) for h in range(HG)]
            seq = [(bi, kt) for bi in range(len(blocks)) for kt in range(KT)]
            sps_tiles = {}
            cps_tiles = {}

            def emit_s(i):
                bi, kt = seq[i]
                qh, h = blocks[bi]
                m, off = h // 2, (h % 2) * 64
                q0 = qh * QW
                s = pp_s.tile([128, QW], F32, name="sps", tag=f"s{i % 2}")
                sps_tiles[i] = s
                for sc in range(QW // 512):
                    nc.tensor.matmul(
                        s[:, sc * 512:(sc + 1) * 512],
                        lhsT=kt_sb[m][off:off + 64, kt * 128:(kt + 1) * 128],
                        rhs=qt_sb[m][off:off + 64,
                                     q0 + sc * 512:q0 + (sc + 1) * 512],
                    )

            emit_s(0)
            vproj(0)
            vproj(1)
            for i, (bi, kt) in enumerate(seq):
                qh, h = blocks[bi]
                m, off = h // 2, (h % 2) * 64
                q0 = qh * QW
                if kt == 0:
                    cps_tiles[bi] = pp_ctx.tile([65, QW], F32, name="cps",
                                                tag="cps")
                cps = cps_tiles[bi]
                s = sps_tiles.pop(i)
                es = espool.tile([128, QW], BF16, name="es", tag="es")
                nc.scalar.activation(es[:, :], s[:, :], EXP, scale=0.125)
                if i + 1 < len(seq):
                    emit_s(i + 1)
                for sc in range(QW // 512):
                    nc.tensor.matmul(
                        cps[:, sc * 512:(sc + 1) * 512],
                        lhsT=vaug_t[kt][:, h * 65:h * 65 + 65],
                        rhs=es[:, sc * 512:(sc + 1) * 512],
                        start=(kt == 0),
                        stop=(kt == KT - 1),
                    )
                if kt == KT - 1:
                    # normalization: evacuate ctx + Z (releases cps), then
                    # reciprocal + partition-broadcast + multiply off the
                    # critical path (DVE/GpSimd/ACT; ACT has slack here)
                    ub = ubpool.tile([64, QW], F32, name="ub", tag="ub")
                    nc.vector.tensor_copy(ub[:, :], cps[0:64, :])
                    zrow = rzpool.tile([1, QW], F32, name="zrow", tag="zrow")
                    nc.scalar.copy(zrow[:, :], cps[64:65, :])
                    rz = rzpool.tile([1, QW], F32, name="rz", tag="rz")
                    with nc.allow_low_precision(reason="~18-bit recip ok"):
                        nc.vector.reciprocal_approx_fast(rz[:, :], zrow[:, :])
                    bsb = bcpool.tile([64, QW], F32, name="bsb", tag="bc")
                    nc.gpsimd.partition_broadcast(bsb[:, :], rz[:, :])
                    nc.vector.tensor_mul(
                        ctx_sb[m][off:off + 64, q0:q0 + QW],
                        ub[:, :],
                        bsb[:, :],
                    )
                    del cps_tiles[bi]
                for fn in bg[i]:
                    fn()

            # ---- out-proj q-half 1 (tail; ACT idle -> alternate engines) ----
            for qt in range(8, 16):
                for n2 in range(2):
                    oproj(qt, n2, on_act=(n2 == 1))

    nc.compile()
    return nc


def kernel(q, k, v, Wq, bq, Wk, bk, Wv, bv, Wo, bo, **extra):
    q = np.asarray(q, np.float32)
    k = np.asarray(k, np.float32)
    v = np.asarray(v, np.float32)
    Wq, Wk, Wv, Wo = (np.asarray(a, np.float32) for a in (Wq, Wk, Wv, Wo))
    bq, bk, bv, bo = (np.asarray(a, np.float32) for a in (bq, bk, bv, bo))
    B = q.shape[0]
    assert q.shape == (B, T, D)

    with_qkv_bias = bool(np.any(bq) or np.any(bk) or np.any(bv))
    if with_qkv_bias not in _NC_CACHE:
        _NC_CACHE[with_qkv_bias] = _build(with_qkv_bias)
    nc = _NC_CACHE[with_qkv_bias]

    bf = ml_dtypes.bfloat16

    def pack_x(x):
        # [T, D] -> x^T [D, T] -> [kb*128, dd*512]: row kb*128+p, col dd*512+c
        a = x.T.reshape(DC, 128, KB, 512).transpose(2, 1, 0, 3)
        return np.ascontiguousarray(a.reshape(KB * 128, DC * 512).astype(bf))

    def pack_w(w):
        # [D, GC] -> [128, DC*GC] chunk-major
        a = w.reshape(DC, 128, GC).transpose(1, 0, 2)
        return np.ascontiguousarray(a.reshape(128, DC * GC).astype(bf))

    xT = {}
    for b in range(B):
        xT[("q", b)] = pack_x(q[b])
        xT[("k", b)] = pack_x(k[b])
        xT[("v", b)] = pack_x(v[b])

    in_maps = []
    for c in range(N_CORES):
        b, g = c // HG, c % HG
        sl = slice(g * GC, (g + 1) * GC)
        m = {
            "xqT": xT[("q", b)],
            "xkT": xT[("k", b)],
            "xvT": xT[("v", b)],
            "wq": pack_w(Wq[:, sl]),
            "wk": pack_w(Wk[:, sl]),
            "wv": pack_w(Wv[:, sl]),
            "wo": np.ascontiguousarray(
                Wo[sl, :].reshape(2, 128, D).transpose(1, 0, 2)
                .reshape(128, 2 * D).astype(bf)),
        }
        if with_qkv_bias:
            m["bqkv"] = np.ascontiguousarray(np.stack([bq[sl], bk[sl], bv[sl]]).astype(bf))
        in_maps.append(m)

    trace = bool(int(os.environ.get("MHA_TRACE", "0")))
    res = run_bass_kernel_spmd(nc, in_maps, list(range(N_CORES)), trace=trace)
    if trace:
        kernel.last_results = res

    out = np.empty((B, T, D), np.float32)
    for b in range(B):
        acc = res.results[b * HG]["out_partial"].astype(np.float32)
        for g in range(1, HG):
            acc = acc + res.results[b * HG + g]["out_partial"]
        out[b] = acc + bo[None, :]
    return out
